# revision 1
# baseline (speedup 1.0000x reference)
"""Contrastive loss kernel for Trainium2 (8 NeuronCores, SPMD data-parallel).

loss = mean_b[ log(exp(pos_b/T) + sum_c exp(neg_bc/T)) - pos_b/T ]
  pos = rowwise dot(z_i, z_j), neg = z_i @ z_k.T, z_* = L2-normalized rows.

Sharding: batch dim of emb_i/emb_j split across 8 cores (1024 rows each);
emb_k replicated. Each core returns per-partition partial sums; host sums.

Per-core device pipeline (all math on device; host only reshapes/shards):
  - emb_k.T uploaded [256, 8192] f32; cast bf16, squared, column-summed via
    ones-matmul on PE -> sumsq_k; r_k = exp(-0.5*ln(sumsq_k)) (one ACT table
    set for Exp+Ln; the banned Rsqrt is avoided); z_kT = e_kT * r_k bcast,
    cast to fp8 with a 16x scale folded into r_k for e4m3 range.
  - lhsT = emb_i_shard.T cast fp8 (raw); 1/(16*T*n_i) folded into the ACT
    per-partition scale at exp time; exp row-sums fused via accum_out.
  - main matmul runs fp8 DoubleRow (both K-halves per instruction).
  - pos/n_i/n_j computed from natural-layout shards via mul+reduce on DVE.
"""

import sys

if "/opt/trn_rl_repo" not in sys.path:
    sys.path.insert(0, "/opt/trn_rl_repo")

import contextlib
import math
import os

import numpy as np

import concourse.bass as bass
import concourse.tile as tile
from concourse import mybir
from concourse.bass import ts
from concourse.bass_utils import run_bass_kernel_spmd
from bass_rust import add_dep_helper

F32 = mybir.dt.float32
BF16 = mybir.dt.bfloat16
AF = mybir.ActivationFunctionType
ALU = mybir.AluOpType

B = 8192          # total batch
D = 256           # embedding dim
NCORES = 8
BP = B // NCORES  # 1024 i-rows per core
NBP = BP // 128   # 8 b-ptiles per core
CHUNK = 1024      # c-columns per prep/main chunk
NCHUNK = B // CHUNK
NKT = D // 128    # 2 contraction tiles
TEMP = 0.5
USE_FP8 = True    # DoubleRow fp8 matmul (2x PE throughput); z scaled by 16
FP8 = mybir.dt.float8e4
ZSCALE = 16.0

_CACHE = {}
ABLATE = os.environ.get("K_ABLATE", "")
DMA_SPLIT = int(os.environ.get("K_DMA_SPLIT", "1"))  # pieces per ekT chunk load

# Max sync-wait conditions this walrus build accepts per instruction.
# Excess waits are moved onto same-engine NoOps inserted before the
# overloaded instruction.
_WAIT_LIMIT_DEFAULT = 1
_WAIT_LIMIT_BY_OPCODE = {"InstEventSemaphore": 2}


def _split_excess_waits(nc):
    n_split = 0
    for fn in nc.m.functions:
        for blk in fn.blocks:
            insts = list(blk.instructions)
            new_insts = []
            for inst in insts:
                si = inst.sync_info
                waits = list(si.on_wait) if si and si.on_wait else []
                lim = _WAIT_LIMIT_BY_OPCODE.get(
                    type(inst).__name__, _WAIT_LIMIT_DEFAULT)
                if len(waits) > lim:
                    excess, keep = waits[:-lim], waits[-lim:]
                    step = max(lim, 1)
                    for i in range(0, len(excess), step):
                        nop = mybir.InstNoOp(
                            name=f"{inst.name}-wsplit-{i}",
                            engine=inst.engine,
                            ins=[], outs=[], debug=inst.debug,
                            sync_info=mybir.SyncInfo(
                                on_wait=excess[i:i + step], on_update=[]),
                        )
                        new_insts.append(nop)
                        n_split += 1
                    inst.sync_info = mybir.SyncInfo(
                        on_wait=keep,
                        on_update=list(si.on_update) if si.on_update else [])
                new_insts.append(inst)
            blk.instructions = new_insts
    return n_split


def _build():
    nc = bass.Bass()
    zdt = FP8 if USE_FP8 else BF16

    ekT = nc.declare_dram_parameter("ekT", [D, B], F32, isOutput=False)
    eiT = nc.declare_dram_parameter("eiT", [D, BP], F32, isOutput=False)
    ei_nat = nc.declare_dram_parameter("ei_nat", [BP, D], F32, isOutput=False)
    ej_nat = nc.declare_dram_parameter("ej_nat", [BP, D], F32, isOutput=False)
    partial = nc.declare_dram_parameter("partial", [128, 1], F32, isOutput=True)
    # DRAM staging for the r_k partition-broadcast (Internal DRAM tensors
    # fail NEFF load under this axon runtime; ExternalOutput works).
    r_scr = nc.declare_dram_parameter("r_scr", [NCHUNK, CHUNK], BF16,
                                      isOutput=True)

    with tile.TileContext(nc) as tc:
        with (
            tc.tile_pool(name="singles", bufs=1) as singles,
            tc.tile_pool(name="zk", bufs=NCHUNK) as zk_pool,
            tc.tile_pool(name="ef32", bufs=4) as ef32_pool,
            tc.tile_pool(name="ebf", bufs=8) as ebf_pool,
            tc.tile_pool(name="sq", bufs=2) as sq_pool,
            tc.tile_pool(name="rbc", bufs=4) as rbc_pool,
            tc.tile_pool(name="row", bufs=2) as row_pool,
            tc.tile_pool(name="small", bufs=6) as small_pool,
            tc.tile_pool(name="mm_psum", bufs=2, space="PSUM") as mm_psum,
            tc.tile_pool(name="prep_psum", bufs=1, space="PSUM") as prep_psum,
            tc.tile_pool(name="tr_psum", bufs=2, space="PSUM") as tr_psum,
        ):
            # ---- constants / residents ----
            ones_bf = singles.tile([128, 1], BF16)
            nc.vector.memset(ones_bf, 1.0)
            ident11 = singles.tile([1, 1], F32)
            nc.vector.memset(ident11, 1.0)
            zero_b = singles.tile([128, 1], F32)
            nc.vector.memset(zero_b, 0.0)
            mlnT_b = singles.tile([128, 1], F32)
            nc.vector.memset(
                mlnT_b,
                -math.log(TEMP) - (math.log(ZSCALE) if USE_FP8 else 0.0))
            lnz_b = singles.tile([128, 1], F32)
            nc.vector.memset(lnz_b, math.log(ZSCALE) if USE_FP8 else 0.0)

            # lhsT: raw emb_i shard, transposed, cast to matmul dtype
            eiT_f = singles.tile([128, NKT, BP], F32)
            nc.gpsimd.dma_start(
                out=eiT_f, in_=eiT[:, :].rearrange("(kt p) b -> p kt b", p=128))
            eiT_c = singles.tile([128, NKT, BP], zdt)
            nc.vector.tensor_copy(eiT_c, eiT_f)

            # ---- i/j natural-layout smalls: sumsq_i, sumsq_j, dot ----
            ei_t = singles.tile([128, NBP, D], F32)
            ej_t = singles.tile([128, NBP, D], F32)
            nc.gpsimd.dma_start(
                out=ei_t, in_=ei_nat[:, :].rearrange("(t p) d -> p t d", p=128))
            nc.gpsimd.dma_start(
                out=ej_t, in_=ej_nat[:, :].rearrange("(t p) d -> p t d", p=128))
            junk = singles.tile([128, D], F32)
            ssi = small_pool.tile([128, NBP], F32)
            ssj = small_pool.tile([128, NBP], F32)
            dot = small_pool.tile([128, NBP], F32)
            ij_insts = []
            for t in range(NBP):
                for dst, a, b in (
                    (ssi, ei_t, ei_t), (ssj, ej_t, ej_t), (dot, ei_t, ej_t),
                ):
                    mi = nc.vector.tensor_mul(junk, a[:, t, :], b[:, t, :])
                    ij_insts.append(mi)
                    nc.vector.tensor_reduce(
                        out=dst[:, t : t + 1], in_=junk,
                        axis=mybir.AxisListType.X, op=ALU.add)

            # s_i = 1/(Z*T*sqrt(ssi)) = exp(-0.5*ln(ssi) - ln(T) - ln(Z))
            s_i = singles.tile([128, NBP], F32)
            r_j = small_pool.tile([128, NBP], F32)
            tmp = small_pool.tile([128, NBP], F32)
            nc.scalar.activation(out=tmp, in_=ssi, func=AF.Ln, bias=zero_b)
            nc.scalar.activation(out=s_i, in_=tmp, func=AF.Exp,
                                 scale=-0.5, bias=mlnT_b)
            nc.scalar.activation(out=tmp, in_=ssj, func=AF.Ln, bias=zero_b)
            nc.scalar.activation(out=r_j, in_=tmp, func=AF.Exp, scale=-0.5,
                                 bias=zero_b)

            # pos_logit = (Z*dot) * s_i * r_j ; exp_pos = exp(pos_logit)
            pos_logit = singles.tile([128, NBP], F32)
            if USE_FP8:
                nc.vector.tensor_scalar_mul(dot, dot, ZSCALE)
            nc.vector.tensor_mul(pos_logit, dot, s_i)
            nc.vector.tensor_mul(pos_logit, pos_logit, r_j)
            exp_pos = singles.tile([128, NBP], F32)
            nc.scalar.activation(out=exp_pos, in_=pos_logit, func=AF.Exp,
                                 bias=zero_b)

            # accumulators for exp row-sums: col index = bp*NCHUNK + cc
            accs = singles.tile([128, NBP * NCHUNK], F32)
            last_main_act = {}  # cc -> last main exp instruction
            zmul_inst = {}      # cc -> z-scale TensorTensor instruction

            # ---- per-chunk prep of z_kT + main matmul/exp ----
            # r_k smalls are batched over BATCH chunks so the ACT stream has
            # few prep ops gating the main exp+accum stream.
            BATCH = 4
            for b0 in range(0, NCHUNK, BATCH):
                bsz = min(BATCH, NCHUNK - b0)
                ebf_list = []
                ss_b = tr_psum.tile([128, bsz * (CHUNK // 128)], F32)
                for ci in range(bsz):
                    cc = b0 + ci
                    ef = ef32_pool.tile([128, NKT, CHUNK], F32)
                    eng = nc.sync if (cc % 2 == 0) else nc.gpsimd
                    eng.dma_start(
                        out=ef,
                        in_=ekT[:, ts(cc, CHUNK)].rearrange(
                            "(kt p) c -> p kt c", p=128))
                    ebf = ebf_pool.tile([128, NKT, CHUNK], BF16)
                    nc.gpsimd.tensor_copy(ebf, ef)
                    ebf_list.append(ebf)

                    sq = sq_pool.tile([128, NKT, CHUNK], BF16)
                    if cc < BATCH:
                        # first squares run on ScalarE (idle during startup;
                        # Square shares the Exp/Ln table set) to keep DVE off
                        # the first batch's critical path
                        sq_inst = nc.scalar.activation(
                            out=sq, in_=ebf, func=AF.Square, bias=zero_b)
                    else:
                        sq_inst = nc.vector.tensor_mul(sq, ebf, ebf)
                    if cc - 4 in zmul_inst:
                        # keep DVE stream interleaved: next batch's squares
                        # must not crowd out older chunks' z-scale muls
                        add_dep_helper(sq_inst.ins, zmul_inst[cc - 4].ins,
                                       reason="order sq after older zmul")

                    # column sums of squares via ones-matmul (accum over kt)
                    ps = prep_psum.tile([1, CHUNK], F32)
                    for kt in range(NKT):
                        for j in range(CHUNK // 512):
                            nc.tensor.matmul(
                                ps[:, ts(j, 512)], ones_bf,
                                sq[:, kt, ts(j, 512)],
                                start=(kt == 0), stop=(kt == NKT - 1))
                    row = row_pool.tile([1, CHUNK], F32)
                    nc.vector.tensor_copy(row, ps)

                    # reshape [1,CHUNK] -> [128, CHUNK//128] via PE
                    # transposes (single-partition SBUF->DRAM DMA fails to
                    # load here, so no DRAM bounce); c = p*(CHUNK//128)+t
                    for t in range(CHUNK // 128):
                        nc.tensor.transpose(
                            ss_b[:, ci * (CHUNK // 128) + t
                                 : ci * (CHUNK // 128) + t + 1],
                            row[0:1, ts(t, 128)], ident11)

                # r_k = Z * exp(-0.5*ln(sumsq)) for the whole batch
                lnt = small_pool.tile([128, bsz * (CHUNK // 128)], F32)
                ln_inst = nc.scalar.activation(out=lnt, in_=ss_b, func=AF.Ln,
                                               bias=zero_b)
                if b0 - 4 in last_main_act:
                    # keep ACT stream interleaved: batch-1 smalls must not be
                    # scheduled ahead of batch-0's main exps
                    add_dep_helper(ln_inst.ins, last_main_act[b0 - 4].ins,
                                   reason="order batch smalls after older mains")
                r_b = small_pool.tile([128, bsz * (CHUNK // 128)], BF16)
                nc.scalar.activation(out=r_b, in_=lnt, func=AF.Exp,
                                     scale=-0.5, bias=lnz_b)
                for ci in range(bsz):
                    nc.sync.dma_start(
                        out=r_scr[b0 + ci, :].rearrange("(p t) -> p t", p=128),
                        in_=r_b[:, ci * (CHUNK // 128)
                                : (ci + 1) * (CHUNK // 128)])

                for ci in range(bsz):
                    cc = b0 + ci
                    # broadcast r over partitions: R[p, c] = r[c]
                    R = rbc_pool.tile([128, CHUNK], BF16)
                    r_ap = r_scr[cc, :]
                    bcast = bass.AP(tensor=r_ap.tensor, offset=r_ap.offset,
                                    ap=[[0, 128]] + list(r_ap.ap))
                    nc.gpsimd.dma_start(out=R, in_=bcast)

                    z = zk_pool.tile([128, NKT, CHUNK], zdt)
                    zmul_inst[cc] = nc.vector.tensor_tensor(
                        z, ebf_list[ci],
                        R[:, None, :].to_broadcast([128, NKT, CHUNK]),
                        ALU.mult)

                    # ---- main: matmul + fused exp/row-sum ----
                    for bp in range(NBP if ABLATE != "nomain" else 0):
                        pt = mm_psum.tile([128, CHUNK], F32)
                        for j in range(CHUNK // 512):
                            nc.tensor.matmul(
                                pt[:, ts(j, 512)],
                                eiT_c[:, :, ts(bp, 128)],
                                z[:, :, ts(j, 512)],
                                perf_mode=mybir.MatmulPerfMode.DoubleRow,
                                start=True, stop=True)
                        nc.scalar.activation(
                            out=pt, in_=pt, func=AF.Exp, bias=zero_b,
                            scale=s_i[:, bp : bp + 1],
                            accum_out=accs[:, bp * NCHUNK + cc
                                           : bp * NCHUNK + cc + 1])

            # ---- epilogue ----
            if ABLATE == "nomain":
                nc.vector.memset(accs, 1.0)
            negsum = small_pool.tile([128, NBP], F32)
            nc.vector.tensor_reduce(
                out=negsum,
                in_=accs[:].rearrange("p (b c) -> p b c", c=NCHUNK),
                axis=mybir.AxisListType.X, op=ALU.add)
            denom = small_pool.tile([128, NBP], F32)
            nc.vector.tensor_add(denom, negsum, exp_pos)
            logd = small_pool.tile([128, NBP], F32)
            nc.scalar.activation(out=logd, in_=denom, func=AF.Ln, bias=zero_b)
            lrows = small_pool.tile([128, NBP], F32)
            nc.vector.tensor_tensor(lrows, logd, pos_logit, ALU.subtract)
            nc.vector.tensor_scalar_mul(lrows, lrows, 1.0 / B)
            rsum = small_pool.tile([128, 1], F32)
            nc.vector.tensor_reduce(out=rsum, in_=lrows,
                                    axis=mybir.AxisListType.X, op=ALU.add)
            nc.sync.dma_start(out=partial[:, :], in_=rsum)

    return nc


def _get_nc(split_waits=False):
    if "nc" not in _CACHE:
        _CACHE["nc"] = _build()
    if split_waits and not _CACHE.get("split"):
        _split_excess_waits(_CACHE["nc"])
        _CACHE["split"] = True
    return _CACHE["nc"]


def _make_in_maps(emb_i, emb_j, emb_k):
    emb_i = np.ascontiguousarray(emb_i, dtype=np.float32)
    emb_j = np.ascontiguousarray(emb_j, dtype=np.float32)
    emb_k = np.ascontiguousarray(emb_k, dtype=np.float32)
    ekT = np.ascontiguousarray(emb_k.T)
    in_maps = []
    for c in range(NCORES):
        sl = slice(c * BP, (c + 1) * BP)
        in_maps.append({
            "ekT": ekT,
            "eiT": np.ascontiguousarray(emb_i[sl].T),
            "ei_nat": np.ascontiguousarray(emb_i[sl]),
            "ej_nat": np.ascontiguousarray(emb_j[sl]),
        })
    return in_maps


def run(emb_i, emb_j, emb_k, trace=False, **kw):
    nc = _get_nc(split_waits=True)
    in_maps = _make_in_maps(emb_i, emb_j, emb_k)
    res = run_bass_kernel_spmd(nc, in_maps, list(range(NCORES)), trace=trace,
                               **kw)
    total = np.float32(0.0)
    for r in res.results:
        total += r["partial"].astype(np.float32).sum()
    return np.float32(total), res


def kernel(emb_i, emb_j, emb_k):
    out, _ = run(emb_i, emb_j, emb_k)
    return out



# revision 57
# speedup vs baseline: 1.3155x; 1.3155x over previous
"""Contrastive loss kernel for Trainium2 (8 NeuronCores, SPMD data-parallel).

loss = mean_b[ log(exp(pos_b/T) + sum_c exp(neg_bc/T)) - pos_b/T ]
  pos = rowwise dot(z_i, z_j), neg = z_i @ z_k.T, z_* = L2-normalized rows.

Sharding: batch dim of emb_i/emb_j split across 8 cores (1024 rows each);
emb_k replicated. Each core returns per-partition partial sums; host sums.

Per-core schedule (ACT is the bottleneck engine: the fused exp+rowsum over
PSUM tiles is ~76us busy; everything else hides under it):
  - emb_k columns are processed in PIECES: a small 256-col bootstrap piece
    plus a 768-col follower, then 1024-col chunks.  The bootstrap piece
    makes the serial startup chain (DMA -> square -> sumsq-matmul -> ln/exp
    -> DRAM bounce -> z-scale -> matmul) short, so the first main exp
    fires at ~10us instead of ~18us.
  - ACT program order is pinned with explicit deps: r_k smalls for piece
    p sit between mains(p-2) and mains(p-1); the greedy list scheduler
    otherwise hoists prep smalls and head-of-line blocks the in-order ACT
    queue.  Bulk ef loads are held behind earlier bounce DMAs on the same
    queue for the same reason.
  - sumsq_i runs as ACT Square+accum in the startup idle gap; sumsq_j /
    dot(i,j) run as DVE mul+reduce mid-kernel, off the critical path.
  - z_k prep: square(ef) on DVE; per-128-col-block sumsq lands directly
    in [128, W/128] layout via tiny PE matmuls (stationary = sq block,
    moving = ones column); ln/exp smalls on ACT; partition broadcast via a
    DRAM bounce; z = ef * R cast to fp8 on DVE (16x scale folded into r_k
    for e4m3 range; 1/(16*T*|e_i|) folded into the ACT per-partition
    scale at exp time; most exp row-sums fused via accum_out, some done
    as DVE tensor_reduce from PSUM to shave the ACT accumulator read).
  - main matmul runs fp8 DoubleRow (both K-halves per instruction).
"""

import sys

if "/opt/trn_rl_repo" not in sys.path:
    sys.path.insert(0, "/opt/trn_rl_repo")

import math

import numpy as np

import concourse.bass as bass
import concourse.tile as tile
from concourse import mybir
from concourse.bass_types import DynSlice
from concourse.bass_utils import run_bass_kernel_spmd
from bass_rust import add_dep_helper

F32 = mybir.dt.float32
BF16 = mybir.dt.bfloat16
AF = mybir.ActivationFunctionType
ALU = mybir.AluOpType

B = 8192          # total batch
D = 256           # embedding dim
NCORES = 8
BP = B // NCORES  # 1024 i-rows per core
NBP = BP // 128   # 8 b-ptiles per core
NKT = D // 128    # 2 contraction tiles
TEMP = 0.5
FP8 = mybir.dt.float8e4
ZSCALE = 16.0

# emb_k column pieces: (start, width); bootstrap 256+768, then 1024s
PIECES = [(0, 256), (256, 768)] + [(k * 1024, 1024) for k in range(1, 8)]
NP = len(PIECES)

_CACHE = {}

# Max sync-wait conditions this walrus build accepts per instruction.
# Excess waits are moved onto same-engine NoOps inserted before the
# overloaded instruction.
_WAIT_LIMIT_DEFAULT = 1
_WAIT_LIMIT_BY_OPCODE = {"InstEventSemaphore": 2}


def _split_excess_waits(nc):
    n_split = 0
    for fn in nc.m.functions:
        for blk in fn.blocks:
            insts = list(blk.instructions)
            new_insts = []
            for inst in insts:
                si = inst.sync_info
                waits = list(si.on_wait) if si and si.on_wait else []
                lim = _WAIT_LIMIT_BY_OPCODE.get(
                    type(inst).__name__, _WAIT_LIMIT_DEFAULT)
                if len(waits) > lim:
                    excess, keep = waits[:-lim], waits[-lim:]
                    step = max(lim, 1)
                    for i in range(0, len(excess), step):
                        nop = mybir.InstNoOp(
                            name=f"{inst.name}-wsplit-{i}",
                            engine=inst.engine,
                            ins=[], outs=[], debug=inst.debug,
                            sync_info=mybir.SyncInfo(
                                on_wait=excess[i:i + step], on_update=[]),
                        )
                        new_insts.append(nop)
                        n_split += 1
                    inst.sync_info = mybir.SyncInfo(
                        on_wait=keep,
                        on_update=list(si.on_update) if si.on_update else [])
                new_insts.append(inst)
            blk.instructions = new_insts
    return n_split


def _build():
    nc = bass.Bass()

    ekT = nc.declare_dram_parameter("ekT", [D, B], F32, isOutput=False)
    eiT = nc.declare_dram_parameter("eiT", [D, BP], F32, isOutput=False)
    ei_nat = nc.declare_dram_parameter("ei_nat", [BP, D], F32, isOutput=False)
    ej_nat = nc.declare_dram_parameter("ej_nat", [BP, D], F32, isOutput=False)
    partial = nc.declare_dram_parameter("partial", [128, 1], F32, isOutput=True)
    # DRAM staging for the r_k partition-broadcast (Internal DRAM tensors
    # fail NEFF load under this axon runtime; ExternalOutput works).
    r_scr = nc.declare_dram_parameter("r_scr", [B], BF16, isOutput=True)

    with tile.TileContext(nc) as tc:
        with (
            tc.tile_pool(name="singles", bufs=1) as singles,
            tc.tile_pool(name="ef", bufs=5) as ef_pool,
            tc.tile_pool(name="sq", bufs=2) as sq_pool,
            tc.tile_pool(name="zk", bufs=NP) as zk_pool,
            tc.tile_pool(name="rbc", bufs=3) as rbc_pool,
            tc.tile_pool(name="small", bufs=8) as small_pool,
            tc.tile_pool(name="mm_psum", bufs=3, space="PSUM") as mm_psum,
            tc.tile_pool(name="tr_psum", bufs=2, space="PSUM") as tr_psum,
        ):
            # ---- constants (DVE) ----
            ones_bf = singles.tile([128, 1], BF16)
            nc.vector.memset(ones_bf, 1.0)
            ident11 = singles.tile([1, 1], F32)
            nc.vector.memset(ident11, 1.0)
            zero_b = singles.tile([128, 1], F32)
            nc.vector.memset(zero_b, 0.0)
            mlnT_b = singles.tile([128, 1], F32)
            nc.vector.memset(mlnT_b, -math.log(TEMP) - math.log(ZSCALE))
            lnz_b = singles.tile([128, 1], F32)
            nc.vector.memset(lnz_b, math.log(ZSCALE))

            # ---- ACT table primer: load the Exp/Ln set at t=0 ----
            prim = singles.tile([128, 1], F32)
            primer = nc.scalar.activation(out=prim, in_=zero_b, func=AF.Exp,
                                          bias=zero_b)

            # ---- startup DMAs (DMA queues exist on SP, Pool, ACT only) ----
            ef_t = [None] * NP

            def load_ef(p, eng, after=None):
                start, w = PIECES[p]
                ef = ef_pool.tile([128, NKT, w], F32)
                di = eng.dma_start(
                    out=ef,
                    in_=ekT[:, DynSlice(start, w)].rearrange(
                        "(kt p) c -> p kt c", p=128))
                if after is not None:
                    # the greedy list scheduler otherwise runs bulk ef loads
                    # ahead of the latency-critical r-bounce DMAs
                    add_dep_helper(di.ins, after.ins,
                                   reason="ef load after bounce DMA")
                ef_t[p] = ef

            load_ef(0, nc.sync)
            load_ef(1, nc.sync)
            # eiT on the ACT queue (idle after the primer) so SP is free for
            # the latency-critical piece-0 bounce DMA
            eiT_f = singles.tile([128, NKT, BP], F32)
            eiT_dma = nc.scalar.dma_start(
                out=eiT_f, in_=eiT[:, :].rearrange("(kt p) b -> p kt b", p=128))
            add_dep_helper(eiT_dma.ins, primer.ins, reason="primer first")
            ei_t = singles.tile([128, NBP, D], F32)
            nc.gpsimd.dma_start(
                out=ei_t, in_=ei_nat[:, :].rearrange("(t p) d -> p t d", p=128))

            # ---- DVE: squares of bootstrap pieces, lhsT cast, ssi ----
            sq_t = [None] * NP
            zmul_inst = [None] * NP

            def emit_sq(p):
                _, w = PIECES[p]
                sq = sq_pool.tile([128, NKT, w], BF16)
                nc.vector.tensor_mul(sq, ef_t[p], ef_t[p])
                sq_t[p] = sq

            emit_sq(0)
            emit_sq(1)

            eiT_c = singles.tile([128, NKT, BP], FP8)
            nc.vector.tensor_copy(eiT_c, eiT_f)

            # (these smalls live across many iterations: persistent pool)
            junk = singles.tile([128, D], F32)
            ssi = singles.tile([128, NBP], F32)
            ssj = singles.tile([128, NBP], F32)
            dot = singles.tile([128, NBP], F32)

            # ---- per-piece prep helpers ----
            tr_last = [None] * NP

            def emit_ps_tr(p, row_after=None):
                start, w = PIECES[p]
                # sumsq columns DIRECTLY in [128, w/128] layout: per
                # 128-col block t, contract the partition (d) axis with a
                # ones column as the moving operand:
                #   ss[m, t] = sum_d sq[d, t*128+m]
                # (stationary = sq block, moving = ones [128,1]); this
                # avoids the [1,w] row + PSUM evacuation + PE transposes.
                ss = tr_psum.tile([128, w // 128], F32)
                for t in range(w // 128):
                    for kt in range(NKT):
                        tr_last[p] = nc.tensor.matmul(
                            ss[:, t : t + 1],
                            sq_t[p][:, kt, DynSlice(t * 128, 128)],
                            ones_bf,
                            start=(kt == 0), stop=(kt == NKT - 1))
                return ss

            r_b_t = [None] * NP
            exp_small = [None] * NP
            main_last = [None] * NP
            main_anchor = [None] * NP

            def emit_smalls(p, ss):
                _, w = PIECES[p]
                # r_k = Z * exp(-0.5*ln(sumsq)) on [128, w/128]
                lnt = small_pool.tile([128, w // 128], F32)
                li = nc.scalar.activation(out=lnt, in_=ss, func=AF.Ln,
                                          bias=zero_b)
                if p >= 2 and main_anchor[p - 2] is not None:
                    # pin ACT order: smalls(p) land early in mains(p-2),
                    # giving the bounce+zmul chain ~1.8 windows of slack;
                    # without a pin the scheduler hoists them and the
                    # in-order ACT queue head-of-line blocks on prep
                    add_dep_helper(li.ins, main_anchor[p - 2].ins,
                                   reason="smalls after older mains")
                r_b = small_pool.tile([128, w // 128], BF16)
                ei = nc.scalar.activation(out=r_b, in_=lnt, func=AF.Exp,
                                          scale=-0.5, bias=lnz_b)
                exp_small[p] = ei
                r_b_t[p] = r_b

            rscr_dma = [None] * NP

            def emit_rscr(p):
                start, w = PIECES[p]
                rscr_dma[p] = nc.sync.dma_start(
                    out=r_scr[DynSlice(start, w)].rearrange(
                        "(p t) -> p t", p=128),
                    in_=r_b_t[p])

            R_t = [None] * NP
            R_dma = [None] * NP

            def emit_R(p):
                start, w = PIECES[p]
                # broadcast r over partitions: R[q, c] = r[c]
                Rt = rbc_pool.tile([128, w], BF16)
                r_ap = r_scr[DynSlice(start, w)]
                bcast = bass.AP(tensor=r_ap.tensor, offset=r_ap.offset,
                                ap=[[0, 128]] + list(r_ap.ap))
                R_dma[p] = nc.gpsimd.dma_start(out=Rt, in_=bcast)
                R_t[p] = Rt

            z_t = [None] * NP

            def emit_zmul(p):
                _, w = PIECES[p]
                z = zk_pool.tile([128, NKT, w], FP8)
                zmul_inst[p] = nc.vector.tensor_tensor(
                    z, ef_t[p],
                    R_t[p][:, None, :].to_broadcast([128, NKT, w]),
                    ALU.mult)
                z_t[p] = z

            # accumulators for exp row-sums: col index = bp*NP + p
            accs = singles.tile([128, NBP * NP], F32)
            s_i = singles.tile([128, NBP], F32)

            # exp row-sum engine per bp: ACT keeps accum_out (187ns aux);
            # DVE does a separate tensor_reduce from PSUM for four bp per
            # piece, shaving ACT time (gpsimd can't reduce the free axis)
            ACC_ENG = ["act", "act", "dve", "act", "act", "act", "dve",
                       "act"]

            def emit_mains(p):
                _, w = PIECES[p]
                for bp in range(NBP):
                    pt = mm_psum.tile([128, w], F32)
                    for j0 in range(0, w, 512):
                        jw = min(512, w - j0)
                        mi = nc.tensor.matmul(
                            pt[:, DynSlice(j0, jw)],
                            eiT_c[:, :, DynSlice(bp * 128, 128)],
                            z_t[p][:, :, DynSlice(j0, jw)],
                            perf_mode=mybir.MatmulPerfMode.DoubleRow,
                            start=True, stop=True)
                        if bp == 0 and j0 == 0 and p + 1 < NP and \
                                tr_last[p + 1] is not None:
                            # pin PE order: piece p+1's transposes must not
                            # queue behind this zmul-gated matmul
                            add_dep_helper(mi.ins, tr_last[p + 1].ins,
                                           reason="mains after next tr")
                    acol = accs[:, bp * NP + p : bp * NP + p + 1]
                    eng = ACC_ENG[bp]
                    if eng == "act":
                        main_last[p] = nc.scalar.activation(
                            out=pt, in_=pt, func=AF.Exp, bias=zero_b,
                            scale=s_i[:, bp : bp + 1], accum_out=acol)
                    else:
                        main_last[p] = nc.scalar.activation(
                            out=pt, in_=pt, func=AF.Exp, bias=zero_b,
                            scale=s_i[:, bp : bp + 1])
                        nc.vector.tensor_reduce(
                            out=acol, in_=pt, axis=mybir.AxisListType.X,
                            op=ALU.add)
                    if bp == 3:
                        main_anchor[p] = main_last[p]

            # ---- piece 0 prep + s_i ----
            ss0 = emit_ps_tr(0)
            emit_smalls(0, ss0)
            emit_rscr(0)
            emit_R(0)
            # ssi via ACT Square+accum: fills ACT's startup idle gap (same
            # table set as Exp/Ln) and keeps DVE free for sq/cast/zmul;
            # ordered after exp0 so it can't head-of-line block the
            # piece-0 bounce chain
            for t in range(NBP):
                sqi = nc.scalar.activation(
                    out=junk, in_=ei_t[:, t, :], func=AF.Square, bias=zero_b,
                    accum_out=ssi[:, t : t + 1])
                if t == 0:
                    add_dep_helper(sqi.ins, exp_small[0].ins,
                                   reason="ssi after piece0 smalls")
            # s_i = 1/(Z*T*sqrt(ssi)) = exp(-0.5*ln(ssi) - ln(T) - ln(Z))
            tmp_i = small_pool.tile([128, NBP], F32)
            nc.scalar.activation(out=tmp_i, in_=ssi, func=AF.Ln, bias=zero_b)
            nc.scalar.activation(out=s_i, in_=tmp_i, func=AF.Exp,
                                 scale=-0.5, bias=mlnT_b)
            emit_zmul(0)

            # ---- piece 1 prep ----
            ss1 = emit_ps_tr(1)
            emit_smalls(1, ss1)
            emit_rscr(1)
            ej_t = singles.tile([128, NBP, D], F32)
            ej_dma = nc.sync.dma_start(
                out=ej_t, in_=ej_nat[:, :].rearrange("(t p) d -> p t d", p=128))
            add_dep_helper(ej_dma.ins, rscr_dma[1].ins,
                           reason="ej load after bounce DMA")
            load_ef(2, nc.gpsimd, after=R_dma[0])
            emit_R(1)
            emit_sq(2)
            emit_zmul(1)

            # ---- steady state: mains(m) with prep(m+2) interleaved ----
            pos_logit = singles.tile([128, NBP], F32)
            exp_pos = singles.tile([128, NBP], F32)
            r_j = singles.tile([128, NBP], F32)

            # ef(m+3) must be in flight by iteration m (sq(m+3) is emitted
            # in iteration m's prep block); alternate SP/Pool queues.  SP
            # loads wait behind an earlier bounce DMA; Pool loads wait for
            # the row copies just emitted, so they can't starve either.
            ef_eng = {3: nc.gpsimd, 4: nc.sync, 5: nc.gpsimd,
                      6: nc.sync, 7: nc.gpsimd, 8: nc.sync}

            for m in range(NP):
                emit_mains(m)
                if m == 0:
                    # ssj / dot smalls: off the critical path; fill DVE gaps
                    # while piece-2's R broadcast is still in flight.
                    # (plain mul+reduce pairs: tensor_tensor_reduce is an
                    # InstISA op this walrus build cannot encode)
                    for t in range(NBP):
                        for dst, a, b in ((ssj, ej_t, ej_t),
                                          (dot, ei_t, ej_t)):
                            nc.vector.tensor_mul(junk, a[:, t, :], b[:, t, :])
                            nc.vector.tensor_reduce(
                                out=dst[:, t : t + 1], in_=junk,
                                axis=mybir.AxisListType.X, op=ALU.add)
                pc = m + 2  # prep piece
                if pc < NP:
                    ssn = emit_ps_tr(pc)
                    emit_smalls(pc, ssn)
                    emit_rscr(pc)
                    emit_R(pc)
                    if pc + 1 < NP:
                        eng = ef_eng[pc + 1]
                        gate = R_dma[pc - 1] if eng is nc.gpsimd \
                            else rscr_dma[pc - 1]
                        load_ef(pc + 1, eng, after=gate)
                        emit_sq(pc + 1)
                    emit_zmul(pc)
                if m == 1:
                    # r_j = exp(-0.5*ln(ssj)); pos_logit = Z*dot*s_i*r_j
                    tmp_j = singles.tile([128, NBP], F32)
                    tji = nc.scalar.activation(out=tmp_j, in_=ssj, func=AF.Ln,
                                               bias=zero_b)
                    add_dep_helper(tji.ins, main_last[1].ins,
                                   reason="r_j smalls after mains1")
                    nc.scalar.activation(out=r_j, in_=tmp_j, func=AF.Exp,
                                         scale=-0.5, bias=zero_b)
                    nc.vector.tensor_scalar_mul(dot, dot, ZSCALE)
                    nc.vector.tensor_mul(pos_logit, dot, s_i)
                    nc.vector.tensor_mul(pos_logit, pos_logit, r_j)
                if m == 2:
                    epi = nc.scalar.activation(out=exp_pos, in_=pos_logit,
                                               func=AF.Exp, bias=zero_b)
                    add_dep_helper(epi.ins, main_last[2].ins,
                                   reason="exp_pos after mains2")

            # ---- epilogue ----
            negsum = small_pool.tile([128, NBP], F32)
            nc.vector.tensor_reduce(
                out=negsum,
                in_=accs[:].rearrange("p (b c) -> p b c", c=NP),
                axis=mybir.AxisListType.X, op=ALU.add)
            denom = small_pool.tile([128, NBP], F32)
            nc.vector.tensor_add(denom, negsum, exp_pos)
            logd = small_pool.tile([128, NBP], F32)
            nc.scalar.activation(out=logd, in_=denom, func=AF.Ln, bias=zero_b)
            lrows = small_pool.tile([128, NBP], F32)
            nc.vector.tensor_tensor(lrows, logd, pos_logit, ALU.subtract)
            nc.vector.tensor_scalar_mul(lrows, lrows, 1.0 / B)
            rsum = small_pool.tile([128, 1], F32)
            nc.vector.tensor_reduce(out=rsum, in_=lrows,
                                    axis=mybir.AxisListType.X, op=ALU.add)
            nc.sync.dma_start(out=partial[:, :], in_=rsum)

    return nc


def _get_nc(split_waits=False):
    if "nc" not in _CACHE:
        _CACHE["nc"] = _build()
    if split_waits and not _CACHE.get("split"):
        _split_excess_waits(_CACHE["nc"])
        _CACHE["split"] = True
    return _CACHE["nc"]


def _make_in_maps(emb_i, emb_j, emb_k):
    emb_i = np.ascontiguousarray(emb_i, dtype=np.float32)
    emb_j = np.ascontiguousarray(emb_j, dtype=np.float32)
    emb_k = np.ascontiguousarray(emb_k, dtype=np.float32)
    ekT = np.ascontiguousarray(emb_k.T)
    in_maps = []
    for c in range(NCORES):
        sl = slice(c * BP, (c + 1) * BP)
        in_maps.append({
            "ekT": ekT,
            "eiT": np.ascontiguousarray(emb_i[sl].T),
            "ei_nat": np.ascontiguousarray(emb_i[sl]),
            "ej_nat": np.ascontiguousarray(emb_j[sl]),
        })
    return in_maps


def run(emb_i, emb_j, emb_k, trace=False, **kw):
    nc = _get_nc(split_waits=True)
    in_maps = _make_in_maps(emb_i, emb_j, emb_k)
    res = run_bass_kernel_spmd(nc, in_maps, list(range(NCORES)), trace=trace,
                               **kw)
    total = np.float32(0.0)
    for r in res.results:
        total += r["partial"].astype(np.float32).sum()
    return np.float32(total), res


def kernel(emb_i, emb_j, emb_k):
    out, _ = run(emb_i, emb_j, emb_k)
    return out
